# revision 2
# baseline (speedup 1.0000x reference)
"""Trainium2 Bass kernel for nn_DecoderAMRPALayer (B=2, S=2048, E=2048, d_k=128).

Sharding: 8 cores = 2 batches x 4 row-chunks of 512 query rows. Each core's
hidden input is row-rotated so its 512 local rows come first; the attention
key/value axis is then a (consistent) permutation of positions, which softmax
and the j-contractions are invariant to.

v7: host-side weight fusion collapses the projection chains:
  - P = hid_loc @ (Wq Wk^T) + bq Wk^T   (one stage instead of Q-then-P;
    P^T tiles land directly in SBUF, no spill/reload; Q itself is only
    needed for its first 128 columns -> tiny separate weight slice)
  - out = H @ (Wv Wp) + b_out           (kills the ctxu stage)
  Scores path f32r, context path bf16. Blocked DRAM layouts (>=4KB rows),
  streams split across both HWDGE queues, hidT j4=1..3 on the gpsimd queue.

Per-core math:
  P^T = Wqk-contraction with hidT[:, :512];  scores = P @ hid^T
  qcam = (hid_loc @ Wq[:, :128] + bq[:128])^T
  K_cam^T = (hid @ Wk[:, :128])^T ;  V_cam = hid @ Wv[:, :128]
  baseA = softmax(SCALE * Qcam @ K_cam^T)
  camctx^T = V_cam^T(via transpose) @ baseA^T
  T = lw * tanh(gate*camctx^T + gate*bv_cam)
  A = softmax(SCALE * (scores + T^T @ K_cam^T))
  H^T = hid^T @ A^T ;  out = H @ Wvp + (bv@Wp + bp)
"""

import sys

sys.path.insert(0, "/opt/trn_rl_repo")

import numpy as np

import concourse.bass as bass
import concourse.mybir as mybir
from concourse import bacc
from concourse.bass import ts
from concourse.bass_utils import run_bass_kernel_spmd
from concourse.masks import make_identity
from concourse.tile import TileContext

F32 = mybir.dt.float32
F32R = mybir.dt.float32r
BF16 = mybir.dt.bfloat16
AF = mybir.ActivationFunctionType
AX = mybir.AxisListType
ALU = mybir.AluOpType

S = 2048
E = 2048
LOC = 512  # local query rows per core
DK = 128
NT = E // 128  # 16 partition tiles
SCALE = 1.0 / float(np.sqrt(128.0))
P = 128


def build():
    nc = bacc.Bacc("TRN2", target_bir_lowering=False, debug=False)

    # f32r inputs (scores path), host-blocked for wide DMA rows
    hidT = nc.dram_tensor("hidT", [16 * P, S], F32R,
                          kind="ExternalInput").ap()   # [(k4,j4,p), kk*512+c]
    wqk = nc.dram_tensor("wqk", [16 * P, S], F32R,
                         kind="ExternalInput").ap()    # [(g,m4,p), kk*512+c]
    wqc = nc.dram_tensor("wqc", [4 * P, 512], F32R,
                         kind="ExternalInput").ap()    # [(k4,p), t*128+d]
    wkc = nc.dram_tensor("wkc", [4 * P, 512], F32R,
                         kind="ExternalInput").ap()
    wvc = nc.dram_tensor("wvc", [4 * P, 512], F32R,
                         kind="ExternalInput").ap()
    # bf16 inputs (context path), blocked [(g,m4,p), kk*512+c] with g=k//4
    hidb = nc.dram_tensor("hidb", [8 * P, 2 * S], BF16,
                          kind="ExternalInput").ap()
    wvp = nc.dram_tensor("wvp", [8 * P, 2 * S], BF16,
                         kind="ExternalInput").ap()
    # small vectors packed host-side: [128, 20] f32
    # cols 0:16 bqk (p,m), 16 bqc, 17 gate, 18 gateb, 19 lw
    smalls = nc.dram_tensor("smalls", [P, 20], F32,
                            kind="ExternalInput").ap()
    bo = nc.dram_tensor("bo", [E], BF16, kind="ExternalInput").ap()
    out = nc.dram_tensor("out", [LOC, E], F32, kind="ExternalOutput").ap()

    with TileContext(nc) as tc:
        with (
            tc.tile_pool(name="const", bufs=1) as pconst,
            tc.tile_pool(name="atp", bufs=1) as pat,
            tc.tile_pool(name="dram", bufs=1, space="DRAM") as pdram,
            tc.tile_pool(name="psA", bufs=1, space="PSUM") as psA,
            tc.tile_pool(name="psB", bufs=1, space="PSUM") as psB,
        ):
            ident_f = pconst.tile([P, P], F32, tag="identf")
            make_identity(nc, ident_f)
            ident_b = pconst.tile([P, P], BF16, tag="identb")
            nc.vector.tensor_copy(ident_b, ident_f)
            ones_b = pconst.tile([1, P], BF16, tag="onesb")
            nc.vector.memset(ones_b, 1.0)
            sm_sb = pconst.tile([P, 20], F32, tag="smalls")
            nc.gpsimd.dma_start(sm_sb, smalls)
            bqk_sb = sm_sb[:, 0:16]
            bqc_sb = sm_sb[:, 16:17]
            gate_sb = sm_sb[:, 17:18]
            gateb_sb = sm_sb[:, 18:19]
            lw_sb = sm_sb[:, 19:20]

            # A^T resident across the hidT-free boundary: [16][128,512] bf16
            at = [pat.tile([P, LOC], BF16, tag=f"at{k}", name=f"at{k}")
                  for k in range(NT)]

            thr = pdram.tile([P, 4], F32R, tag="thr")  # h-rest throttle token

            def mm(ps, lhsT, rhs, start, stop):
                nc.tensor.matmul(ps, lhsT, rhs, start=start, stop=stop)

            def softmax_tr(pool, pa, pb, dsts, ex_bufs):
                """psum halves [128,1024]x2 -> normalized exp (bf16) ->
                transposed [128,128] blocks handed to dsts(jq, tp_psum)."""
                ex = pool.tile([P, S], BF16, tag="ex", bufs=ex_bufs, name="ex")
                s1_ = pool.tile([P, 1], F32, tag="s1", bufs=2, name="s1")
                s2_ = pool.tile([P, 1], F32, tag="s2", bufs=2, name="s2")
                nc.scalar.activation(ex[:, 0:1024], pa, AF.Exp,
                                     scale=SCALE, accum_out=s1_)
                nc.scalar.activation(ex[:, 1024:2048], pb, AF.Exp,
                                     scale=SCALE, accum_out=s2_)
                nc.vector.tensor_tensor(s1_, s1_, s2_, op=ALU.add)
                rec = pool.tile([P, 1], F32, tag="rec", bufs=2, name="rec")
                nc.vector.reciprocal(rec, s1_)
                nc.vector.tensor_scalar_mul(ex, ex, rec)
                for jq in range(4):
                    tp = psB.tile([P, 512], BF16, tag="tr", bufs=2, name="tp")
                    for t in range(4):
                        nc.tensor.matmul(
                            tp[:, ts(t, P)], ex[:, ts(jq * 4 + t, P)], ident_b,
                            start=True, stop=True, is_transpose=True,
                            skip_group_check=True)
                    dsts(jq, tp)

            # ---------------- hidT resident: 16 grouped tiles hg[j4][k4]
            with tc.tile_pool(name="hidT", bufs=1) as phid:
                hg = [[phid.tile([P, S], F32R, tag=f"h{j4}_{k4}",
                                 name=f"h{j4}_{k4}") for k4 in range(4)]
                      for j4 in range(4)]

                def hsl(k, j4):
                    return hg[j4][k // 4][:, (k % 4) * 512:(k % 4 + 1) * 512]

                # j4=0 (P^T's rhs) on sync, full tiles (8KB packets win the
                # per-packet DMA arbitration); j4=1..3 issued later, throttled
                for k4 in range(4):
                    nc.sync.dma_start(hg[0][k4], hidT[ts(k4 * 4 + 0, P), :])

                # pcam spans P^T..s7
                with tc.tile_pool(name="cam", bufs=1) as pcam:
                    qcam = pcam.tile([P, LOC], BF16, tag="qcam")
                    kcamR = pcam.tile([P, S], BF16, tag="kcamR")
                    T_sb = pcam.tile([P, LOC], BF16, tag="T")
                    # P^T resident: [4][128,2048], tile g holds m-tiles g*4+j
                    ptg = [pcam.tile([P, S], F32R, tag=f"pt{g}",
                                     name=f"pt{g}") for g in range(4)]

                    def ptsl(k, ic):
                        return ptg[k // 4][:, (k % 4) * 512 + ic * P:
                                           (k % 4) * 512 + (ic + 1) * P]

                    with tc.tile_pool(name="work1", bufs=1) as pwk:
                        # -------- sP: P^T = Wqk-contraction (+bqk bias)
                        def sP_group(m4):
                            pa = psA.tile([P, 1024], F32, tag="big", bufs=2,
                                          name="pa")
                            pb = psA.tile([P, 1024], F32, tag="big", bufs=2,
                                          name="pb")
                            slots = [pa[:, 0:512], pa[:, 512:1024],
                                     pb[:, 0:512], pb[:, 512:1024]]
                            for g in range(4):
                                wg = pwk.tile([P, S], F32R, tag="w_in",
                                              bufs=2, name="wg")
                                dq = nc.scalar if g % 2 else nc.sync
                                dq.dma_start(wg, wqk[ts(g * 4 + m4, P), :])
                                for ks in range(4):
                                    k = g * 4 + ks
                                    for j in range(4):
                                        mm(slots[j],
                                           wg[:, ks * 512 + j * P:
                                              ks * 512 + (j + 1) * P],
                                           hsl(k, 0), k == 0, k == NT - 1)
                            for j in range(4):
                                m = m4 * 4 + j
                                nc.vector.tensor_scalar_add(
                                    ptg[m4][:, j * 512:(j + 1) * 512],
                                    slots[j], bqk_sb[:, m:m + 1])

                        # -------- sQc: qcam = (hid_loc @ Wq[:,:128] + b)^T
                        def sQc():
                            ps = psA.tile([P, 512], F32, tag="mm", bufs=2,
                                          name="ps")
                            for k4 in range(4):
                                wct = pwk.tile([P, 512], F32R, tag="wc",
                                               bufs=2, name="wct")
                                nc.scalar.dma_start(wct, wqc[ts(k4, P), :])
                                for ks in range(4):
                                    k = k4 * 4 + ks
                                    mm(ps, wct[:, ks * P:(ks + 1) * P],
                                       hsl(k, 0), k == 0, k == NT - 1)
                            nc.vector.tensor_scalar_add(qcam, ps, bqc_sb)

                        sP_group(0)
                        # throttle h-rest behind sP(0): the dummy read chains
                        # ptg[0] (sP(0) output) -> hg tile read -> DMA WAR
                        tok = pwk.tile([P, 1], F32R, tag="tok", bufs=1)
                        for k4 in range(4):
                            for j4 in range(1, 4):
                                nc.vector.tensor_tensor(
                                    tok, ptg[0][:, 0:1], hg[j4][k4][:, 0:1],
                                    op=ALU.add)
                                nc.gpsimd.dma_start(
                                    hg[j4][k4], hidT[ts(k4 * 4 + j4, P), :])
                        sP_group(1)
                        sP_group(2)
                        sP_group(3)
                        sQc()

                    with tc.tile_pool(name="work2", bufs=1) as pwk:
                        vnat = pwk.tile([P, S], BF16, tag="vnat")

                        def cam_mm(wsrc, sink):
                            pa = psA.tile([P, 1024], F32, tag="big",
                                          bufs=2, name="pa")
                            pb = psA.tile([P, 1024], F32, tag="big",
                                          bufs=2, name="pb")
                            slots = [pa[:, 0:512], pa[:, 512:1024],
                                     pb[:, 0:512], pb[:, 512:1024]]
                            for k4 in range(4):
                                wct = pwk.tile([P, 512], F32R, tag="wc",
                                               bufs=2, name="wct")
                                nc.scalar.dma_start(wct, wsrc[ts(k4, P), :])
                                for ks in range(4):
                                    k = k4 * 4 + ks
                                    for j4 in range(4):
                                        mm(slots[j4],
                                           wct[:, ks * P:(ks + 1) * P],
                                           hsl(k, j4), k == 0, k == NT - 1)
                            for j4 in range(4):
                                sink(j4, slots[j4])

                        def kc_sink(j4, ps):
                            nc.vector.tensor_copy(kcamR[:, ts(j4, 512)], ps)

                        def vn_sink(j4, ps):
                            vstg = pwk.tile([P, 512], BF16, tag="vstg",
                                            bufs=1, name="vstg")
                            nc.vector.tensor_copy(vstg, ps)
                            tp = psB.tile([P, 512], BF16, tag="tr",
                                          bufs=2, name="tp")
                            for t4 in range(4):
                                nc.tensor.matmul(
                                    tp[:, ts(t4, P)], vstg[:, ts(t4, P)],
                                    ident_b, start=True, stop=True,
                                    is_transpose=True, skip_group_check=True)
                            nc.vector.tensor_copy(vnat[:, ts(j4, 512)], tp)

                        def s5_group(ic):
                            pa = psA.tile([P, 1024], F32, tag="big",
                                          bufs=2, name="pa")
                            pb = psA.tile([P, 1024], F32, tag="big",
                                          bufs=2, name="pb")
                            for hi, ph in enumerate((pa, pb)):
                                for j2 in range(2):
                                    mm(ph[:, ts(j2, 512)], qcam[:, ts(ic, P)],
                                       kcamR[:, ts(hi * 2 + j2, 512)],
                                       True, True)

                            def bA_dst(jq, tp, ic=ic):
                                for t in range(4):
                                    nc.vector.tensor_copy(
                                        at[jq * 4 + t][:, ts(ic, P)],
                                        tp[:, ts(t, P)])

                            softmax_tr(pwk, pa, pb, bA_dst, 1)

                        # s6 camctx groups share ONE psum bank (4 x 128-col
                        # accumulation groups) and run inside the s5 gaps
                        cps = psA.tile([P, 512], F32, tag="mm", bufs=2,
                                       name="cps")

                        def s6_group(ic):
                            for jt in range(NT):
                                mm(cps[:, ts(ic, P)], vnat[:, ts(jt, P)],
                                   at[jt][:, ts(ic, P)], jt == 0,
                                   jt == NT - 1)

                        cam_mm(wkc, kc_sink)
                        s5_group(0)
                        cam_mm(wvc, vn_sink)
                        s6_group(0)
                        s5_group(1)
                        s6_group(1)
                        s5_group(2)
                        s6_group(2)
                        s5_group(3)
                        s6_group(3)
                        ttmp = pwk.tile([P, LOC], F32, tag="ttmp", bufs=1)
                        for ic in range(4):
                            nc.vector.tensor_scalar(
                                ttmp[:, ts(ic, P)], cps[:, ts(ic, P)],
                                gate_sb, gateb_sb, op0=ALU.mult, op1=ALU.add)
                        nc.scalar.activation(ttmp, ttmp, AF.Tanh)
                        nc.vector.tensor_scalar_mul(T_sb, ttmp, lw_sb)

                    # -------- s7: main scores -> A^T resident (bf16)
                    with tc.tile_pool(name="s7w", bufs=1) as ps7:
                        for ic in range(4):
                            pa = psA.tile([P, 1024], F32, tag="big",
                                          bufs=2, name="pa")
                            pb = psA.tile([P, 1024], F32, tag="big",
                                          bufs=2, name="pb")
                            slots = [pa[:, 0:512], pa[:, 512:1024],
                                     pb[:, 0:512], pb[:, 512:1024]]
                            for k in range(NT):
                                for j4 in range(4):
                                    mm(slots[j4], ptsl(k, ic),
                                       hsl(k, j4), k == 0, False)
                            for j4 in range(4):
                                mm(slots[j4], T_sb[:, ts(ic, P)],
                                   kcamR[:, ts(j4, 512)], False, True)

                            def A_dst(jq, tp, ic=ic):
                                for t in range(4):
                                    nc.vector.tensor_copy(
                                        at[jq * 4 + t][:, ts(ic, P)],
                                        tp[:, ts(t, P)])

                            softmax_tr(ps7, pa, pb, A_dst, 2)

            # ---------------- s8 + s10 (hidT freed; bf16 context chain)
            with tc.tile_pool(name="ht", bufs=1) as pht:
                ht = [pht.tile([P, LOC], BF16, tag=f"ht{m}", name=f"ht{m}")
                      for m in range(NT)]
                # s8: H^T = hid^T @ A^T  (lhsT = hid natural bf16 tiles)
                for m4 in range(4):
                    pa = psA.tile([P, 1024], F32, tag="big", bufs=2,
                                  name="pa")
                    pb = psA.tile([P, 1024], F32, tag="big", bufs=2,
                                  name="pb")
                    slots = [pa[:, 0:512], pa[:, 512:1024],
                             pb[:, 0:512], pb[:, 512:1024]]
                    for g2 in range(2):
                        wg = pht.tile([P, 2 * S], BF16, tag="w_in",
                                      bufs=2, name="wg")
                        dqe = nc.scalar if g2 % 2 else nc.sync
                        dqe.dma_start(wg, hidb[ts(g2 * 4 + m4, P), :])
                        for ks in range(8):
                            k = g2 * 8 + ks
                            for j in range(4):
                                mm(slots[j],
                                   wg[:, ks * 512 + j * P:
                                      ks * 512 + (j + 1) * P],
                                   at[k], k == 0, k == NT - 1)
                    for j in range(4):
                        nc.vector.tensor_copy(ht[m4 * 4 + j], slots[j])

                # s10: out = H @ Wvp + b_out  (lhsT = H^T tiles)
                bo_sb = pht.tile([1, E], BF16, tag="bo")
                nc.gpsimd.dma_start(bo_sb, bo.rearrange("(o f) -> o f", o=1))
                for n4 in range(4):
                    pa = psA.tile([P, 1024], F32, tag="big", bufs=2, name="pa")
                    pb = psA.tile([P, 1024], F32, tag="big", bufs=2, name="pb")
                    slots = [pa[:, 0:512], pa[:, 512:1024],
                             pb[:, 0:512], pb[:, 512:1024]]
                    for g2 in range(2):
                        wg = pht.tile([P, 2 * S], BF16, tag="w_in", bufs=2,
                                      name="wg")
                        dqe = nc.scalar if g2 % 2 else nc.sync
                        dqe.dma_start(wg, wvp[ts(g2 * 4 + n4, P), :])
                        for ks in range(8):
                            k = g2 * 8 + ks
                            for ic in range(4):
                                mm(slots[ic], ht[k][:, ts(ic, P)],
                                   wg[:, ks * 512:(ks + 1) * 512], k == 0,
                                   False)
                    for ic in range(4):
                        mm(slots[ic], ones_b, bo_sb[0:1, ts(n4, 512)], False,
                           True)
                        ostg = pht.tile([P, 512], F32, tag="ostg", bufs=2,
                                        name="ostg")
                        nc.vector.tensor_copy(ostg, slots[ic])
                        dqo = nc.gpsimd if ic % 2 else nc.sync
                        dqo.dma_start(out[ts(ic, P), ts(n4, 512)], ostg)

    nc.compile()
    return nc


_NC = None


def _get_nc():
    global _NC
    if _NC is None:
        _NC = build()
    return _NC


def _blk2(w):
    # [E, E] -> [(k2,m4,p), kk2*512+c] flat [32*128, 1024]
    return np.ascontiguousarray(
        w.reshape(8, 2, 128, 4, 512).transpose(0, 3, 2, 1, 4).reshape(
            32 * 128, 1024))


def _blk4(w):
    # [2048, 2048] -> [(g,m4,p), kk*512+c] flat [16*128, 2048], g = row//512
    return np.ascontiguousarray(
        w.reshape(4, 4, 128, 4, 512).transpose(0, 3, 2, 1, 4).reshape(
            16 * 128, 2048))


def _blk8(w):
    # [2048, 2048] -> [(g2,m4,p), kk*512+c] flat [8*128, 4096], g2 = row//1024
    return np.ascontiguousarray(
        w.reshape(2, 8, 128, 4, 512).transpose(0, 3, 2, 1, 4).reshape(
            8 * 128, 4096))


def _blkc(w):
    # [E, DK] -> [(k4,p), t*128+d] flat [4*128, 512]
    return np.ascontiguousarray(
        w.reshape(4, 4, 128, 128).transpose(0, 2, 1, 3).reshape(4 * 128, 512))


def make_in_maps(hidden_states, c_attn_w, c_attn_b, c_proj_w, c_proj_b,
                 cam_gate, cam_w0, cam_w1):
    import ml_dtypes
    hs = np.ascontiguousarray(np.asarray(hidden_states, dtype=np.float32))
    W = np.asarray(c_attn_w, dtype=np.float32)
    b = np.asarray(c_attn_b, dtype=np.float32)
    Wp = np.ascontiguousarray(np.asarray(c_proj_w, dtype=np.float32))
    bp = np.asarray(c_proj_b, dtype=np.float32)
    gate = np.ascontiguousarray(np.asarray(cam_gate, dtype=np.float32))
    w0 = float(np.asarray(cam_w0).reshape(-1)[0])
    w1 = float(np.asarray(cam_w1).reshape(-1)[0])

    wqm = np.ascontiguousarray(W[:, :E])
    wkm = W[:, E:2 * E]
    wvm = W[:, 2 * E:]
    bq = b[:E]
    bv = b[2 * E:].astype(np.float64)

    # host-fused weights
    wqk_b = _blk4(wqm @ np.ascontiguousarray(wkm.T))   # Wq @ Wk^T
    bqk = np.ascontiguousarray(wkm @ bq)               # (bq @ Wk^T)[ei]
    wvp_f = np.ascontiguousarray(wvm) @ Wp             # Wv @ Wp
    wvp_b = _blk8(wvp_f.astype(ml_dtypes.bfloat16))
    wqc = _blkc(np.ascontiguousarray(wqm[:, :DK]))
    wkc = _blkc(np.ascontiguousarray(wkm[:, :DK]))
    wvc = _blkc(np.ascontiguousarray(wvm[:, :DK]))
    bqc = np.ascontiguousarray(bq[:DK])

    lw = 1.0 / (1.0 + np.exp(-(w0 + w1 * 0.5)))
    gateb = gate * b[2 * E:2 * E + DK]
    b_out = (bv @ Wp.astype(np.float64) + bp.astype(np.float64)).astype(
        ml_dtypes.bfloat16)
    smalls = np.zeros((128, 20), np.float32)
    smalls[:, 0:16] = bqk.reshape(16, 128).T
    smalls[:, 16] = bqc
    smalls[:, 17] = gate
    smalls[:, 18] = gateb
    smalls[:, 19] = lw

    in_maps = []
    for c in range(8):
        bi, rr = divmod(c, 4)
        hb = hs[bi]
        hid_roll = np.concatenate([hb[rr * LOC:], hb[:rr * LOC]], axis=0)
        hidT_b = _blk4(np.ascontiguousarray(hid_roll.T))
        hidb_b = _blk8(hid_roll.astype(ml_dtypes.bfloat16))
        in_maps.append({
            "hidT": hidT_b, "wqk": wqk_b, "wqc": wqc, "wkc": wkc, "wvc": wvc,
            "hidb": hidb_b, "wvp": wvp_b, "smalls": smalls, "bo": b_out,
        })
    return in_maps


def kernel(**inputs):
    nc = _get_nc()
    in_maps = make_in_maps(**inputs)
    res = run_bass_kernel_spmd(nc, in_maps, core_ids=list(range(8)))
    out = np.empty((2, S, E), dtype=np.float32)
    for c in range(8):
        bi, rr = divmod(c, 4)
        out[bi, rr * LOC:(rr + 1) * LOC] = res.results[c]["out"]
    return out
